# revision 21
# baseline (speedup 1.0000x reference)
"""MoE (top-2 of 8 experts) SwiGLU FFN on 8 Trainium2 NeuronCores.

Strategy — expert-parallel with K-way H-sharding for load balance:
  - Router (x @ w_gate -> softmax -> top-2) computed host-side on jax-CPU
    with the exact ops the reference uses, so expert selection matches the
    reference bit-for-bit ("dispatch tokens by topk_idx").
  - Each expert's H dimension is split over K cores; the SPMD program runs
    NE*K/8 slots.  Slot k covers 8/K experts (grouped by sorted load so the
    slot cap = group max is tight); core c runs expert group-member c//K
    with H-chunk c%K.  K trades PE-cycle balance (larger K -> caps approach
    the per-core arithmetic floor) against duplicated x traffic and
    partial-output volume (K-fold).  K=2 keeps the instruction structure of
    plain expert-parallel while cutting the cap padding from
    8*max_load to 4*(l1+l5) tokens per core.
  - Device computes, per slot, partial y^T = wo_c^T @ (silu(wg_c^T x^T) *
    (wi_c^T x^T)) with bf16 matmuls accumulating in fp32 PSUM.  Tokens stay
    on the PSUM free dimension throughout, so no on-device transposes are
    needed.  Partial y (this core's H-chunk contribution) is written bf16
    via per-(slot, out-chunk) coalesced strip DMAs (many small output DMAs
    saturate the issuing queue and stall PSUM drain); the host sums the K
    partials per expert in fp32 and applies the top-2 combine weights.
  - Perf notes baked in: chip-level power arbitration throttles the PE
    clock when all 8 cores stream matmuls (2.4 -> ~1.9 GHz), so total PE
    cycles and total chip activity (DMA bytes, DVE element work) both
    matter; warm-up matmuls cover the initial DMA wait and pre-warm the
    HAM clock gate; weight layouts are hh-major so the first matmul group
    needs only one small DMA chunk.
"""

import numpy as np
import ml_dtypes

import concourse.bass as bass
import concourse.mybir as mybir
import concourse.tile as tile
from concourse.bass_utils import run_bass_kernel_spmd

N_CORES = 8
N_EXPERTS = 8
TOP_K = 2
B, T, C, H = 4, 2048, 1024, 2048
CC = C // 128            # contraction chunks over C
TOK_TILE = 512           # max tokens per PSUM tile (one fp32 bank)
CO = C // 128            # output row chunks
BF16 = mybir.dt.bfloat16

KSH = 2                  # H-shard factor (1, 2, 4 or 8)


def _split_multi_waits(nc, max_waits=1):
    """This walrus build rejects >1 sync-wait per instruction. Peel extra
    waits onto single-wait EventSemaphore instructions inserted just before,
    on the same engine (identical blocking semantics)."""
    n_split = 0
    for fn in nc.m.functions:
        for bb in fn.blocks:
            out = []
            changed = False
            for inst in bb.instructions:
                si = inst.sync_info
                waits = list(si.on_wait) if si is not None else []
                if len(waits) > max_waits:
                    head, keep = waits[:-max_waits], waits[-max_waits:]
                    for j, w in enumerate(head):
                        out.append(mybir.InstEventSemaphore(
                            name=f"{inst.name}-wspl{j}",
                            engine=inst.engine,
                            sync_info=mybir.SyncInfo(on_wait=[w], on_update=[]),
                        ))
                    inst.sync_info = mybir.SyncInfo(
                        on_wait=keep, on_update=list(si.on_update))
                    changed = True
                    n_split += 1
                out.append(inst)
            if changed:
                bb.instructions = out
    return n_split


def tok_tiling(cap):
    """Token tiles with a small first tile (fast first x DMA), the rest
    split evenly."""
    if cap <= 256:
        return [(0, cap)]
    first = 128
    rest = cap - first
    k = -(-rest // TOK_TILE)
    w = -(-rest // (4 * k)) * 4
    out = [(0, first)]
    t0 = first
    while t0 < cap:
        tw = min(w, cap - t0)
        out.append((t0, tw))
        t0 += tw
    return out


def tiling_plain(cap):
    """Even split into ceil(cap/512) tiles (multiple-of-4 widths): avoids
    tiny tail tiles whose matmul groups are all fixed overhead."""
    k = -(-cap // TOK_TILE)
    w = -(-cap // (4 * k)) * 4
    out = []
    t0 = 0
    while t0 < cap:
        tw = min(w, cap - t0)
        out.append((t0, tw))
        t0 += tw
    return out


def slot_tiling(j, cap):
    # even tiles everywhere: the PE warm-up matmuls already cover the first
    # x DMA, and a small first tile costs ~256 extra overhead-heavy
    # small-N matmuls across both stages
    return tiling_plain(cap)


def _geom(K):
    hsh = H // K             # H columns per core per slot
    hshh = hsh // 128        # hh chunks per slot
    nslot = N_EXPERTS * K // N_CORES
    w1blk = hshh * CC * 128  # stage-1 weight cols per slot
    w2blk = CO * hshh * 128  # stage-2 weight cols per slot
    return hsh, hshh, nslot, w1blk, w2blk


def build_program(caps, K=KSH, reps=1):
    """SPMD program over nslot slots; slot k = one expert of its group
    (which one depends on core // K), H-chunk = core % K.

    DRAM inputs (bf16), slot-concatenated along columns:
      xtb [128, CC*sum(caps)]   slot k at CC*capoff_k per slot_tiling(k,cap)
      wib/wgb [128, nslot*w1blk]  slot k: [p,(hh*CC+cc)*128+f] =
                                  wi[e][cc*128+p, q*hsh+hh*128+f]
      wob [128, nslot*w2blk]      slot k: [p,(co*hshh+hh)*128+f] =
                                  wo[e][q*hsh+hh*128+p, co*128+f]
    Output ytp [C, sum(caps)] bf16: partial y^T (this core's H-chunk).
    reps>1 repeats the whole compute (timing use only).
    """
    hsh, hshh, nslot, w1blk, w2blk = _geom(K)
    assert len(caps) == nslot
    scap = sum(caps)
    capoff = [sum(caps[:k]) for k in range(nslot)]

    nc = bass.Bass()
    xtb = nc.dram_tensor("xtb", [128, CC * scap], BF16, kind="ExternalInput")
    wib = nc.dram_tensor("wib", [128, nslot * w1blk], BF16,
                         kind="ExternalInput")
    wgb = nc.dram_tensor("wgb", [128, nslot * w1blk], BF16,
                         kind="ExternalInput")
    wob = nc.dram_tensor("wob", [128, nslot * w2blk], BF16,
                         kind="ExternalInput")
    ytp = nc.dram_tensor("ytp", [C, scap], BF16, kind="ExternalOutput")
    # tiny output: fetching it blocks on program completion without paying
    # the big ytp transfer through the tunnel (timing use)
    done = nc.dram_tensor("done", [1, 8], BF16, kind="ExternalOutput")

    with tile.TileContext(nc) as tc:
        with tc.tile_pool(name="wu", bufs=1) as wu_pool, \
             tc.tile_pool(name="xb", bufs=1) as xb_pool, \
             tc.tile_pool(name="w1", bufs=3) as w1_pool, \
             tc.tile_pool(name="hT", bufs=1) as h_pool, \
             tc.tile_pool(name="w2", bufs=3) as w2_pool, \
             tc.tile_pool(name="sg", bufs=3) as sg_pool, \
             tc.tile_pool(name="yo", bufs=1) as yo_pool, \
             tc.tile_pool(name="psw", bufs=1, space="PSUM") as psw_pool, \
             tc.tile_pool(name="ps", bufs=2, space="PSUM") as ps_pool, \
             tc.tile_pool(name="ps2", bufs=3, space="PSUM") as ps2_pool:

            # warm-up: keep PE busy during the first DMAs (also warms the
            # HAM clock gate); runs on a zeroed scratch tile into a PSUM
            # bank nothing reads
            wup = wu_pool.tile([128, TOK_TILE], BF16, tag="wup")
            nc.vector.memset(wup[:], 0)
            ps_w = psw_pool.tile([128, TOK_TILE], mybir.dt.float32, tag="pw")
            for _ in range(14):
                nc.tensor.matmul(ps_w[:], wup[:, 0:128], wup[:],
                                 start=True, stop=True)

            mcap = max(caps)
            for _rep in range(reps):
                for j in range(nslot):
                    cap = caps[j]
                    tiles = slot_tiling(j, cap)
                    xoff = CC * capoff[j]

                    # slot 0: per-tile loads so the first matmul group only
                    # waits on a small transfer; others: one big DMA,
                    # double-buffered so the load runs a slot ahead
                    xs = xb_pool.tile([128, CC * mcap], BF16,
                                      tag=f"x{j % 2}", name=f"xs{j}")
                    if j == 0:
                        off = xoff
                        for t0, tw in tiles:
                            nc.gpsimd.dma_start(
                                xs[:, CC * t0:CC * t0 + CC * tw],
                                xtb[:, off:off + CC * tw])
                            off += CC * tw
                    else:
                        nc.gpsimd.dma_start(xs[:, :CC * cap],
                                            xtb[:, xoff:xoff + CC * cap])
                    xts = [xs[:, CC * t0:CC * (t0 + tw)]
                           for t0, tw in tiles]

                    hT = h_pool.tile([128, hshh * mcap], BF16, tag="hT")

                    # ---- stage 1: hT = silu(x@wg) * (x@wi), H-chunk ----
                    # weights stream in per-hh chunks (3-deep ring): the
                    # first matmul group waits only on one 256 KB chunk,
                    # and slot boundaries never stall on a big reload
                    WCH = CC * 128
                    for hh in range(hshh):
                        wib_t = w1_pool.tile([128, WCH], BF16, tag="wib")
                        nc.sync.dma_start(
                            wib_t[:], wib[:, j * w1blk + hh * WCH:
                                          j * w1blk + (hh + 1) * WCH])
                        wgb_t = w1_pool.tile([128, WCH], BF16, tag="wgb")
                        nc.sync.dma_start(
                            wgb_t[:], wgb[:, j * w1blk + hh * WCH:
                                          j * w1blk + (hh + 1) * WCH])
                        for ti, (t0, tw) in enumerate(tiles):
                            ps_u = ps_pool.tile([128, TOK_TILE],
                                                mybir.dt.float32, tag="psu")
                            ps_g = ps_pool.tile([128, TOK_TILE],
                                                mybir.dt.float32, tag="psg")
                            for cc in range(CC):
                                nc.tensor.matmul(
                                    ps_u[:, :tw],
                                    wib_t[:, cc * 128:(cc + 1) * 128],
                                    xts[ti][:, cc * tw:(cc + 1) * tw],
                                    start=(cc == 0), stop=(cc == CC - 1))
                            for cc in range(CC):
                                nc.tensor.matmul(
                                    ps_g[:, :tw],
                                    wgb_t[:, cc * 128:(cc + 1) * 128],
                                    xts[ti][:, cc * tw:(cc + 1) * tw],
                                    start=(cc == 0), stop=(cc == CC - 1))
                            sg = sg_pool.tile([128, TOK_TILE],
                                              mybir.dt.float32, tag="sg")
                            nc.scalar.activation(
                                sg[:, :tw], ps_g[:, :tw],
                                mybir.ActivationFunctionType.Silu)
                            nc.vector.tensor_mul(
                                hT[:, hh * mcap + t0: hh * mcap + t0 + tw],
                                ps_u[:, :tw], sg[:, :tw])

                    # ---- stage 2: partial y^T = wo_chunk^T @ hT ----
                    W2CH = hshh * 128
                    for co in range(CO):
                        c0 = co * 128
                        wob_t = w2_pool.tile([128, W2CH], BF16, tag="wob")
                        nc.sync.dma_start(
                            wob_t[:], wob[:, j * w2blk + co * W2CH:
                                          j * w2blk + (co + 1) * W2CH])
                        yo = yo_pool.tile([128, mcap], BF16,
                                          tag=f"yo{co % 2}", name=f"yo{co}")
                        for t0, tw in tiles:
                            ps_y = ps2_pool.tile([128, TOK_TILE],
                                                 mybir.dt.float32, tag="psy")
                            for hh in range(hshh):
                                nc.tensor.matmul(
                                    ps_y[:, :tw],
                                    wob_t[:, hh * 128:(hh + 1) * 128],
                                    hT[:, hh * mcap + t0:
                                       hh * mcap + t0 + tw],
                                    start=(hh == 0), stop=(hh == hshh - 1))
                            nc.vector.tensor_copy(yo[:, t0:t0 + tw],
                                                  ps_y[:, :tw])
                        nc.scalar.dma_start(
                            ytp[c0:c0 + 128, capoff[j]:capoff[j] + cap],
                            yo[:, :cap])
                        if j == nslot - 1 and co == CO - 1:
                            nc.scalar.dma_start(done[0:1, 0:8], yo[0:1, 0:8])
    _split_multi_waits(nc)
    return nc


def pack_wi(w_e, q, K=KSH):
    """wi/wg [C, H] f32, H-chunk q of K -> [128, w1blk] bf16, hh-major."""
    hsh, hshh, _, w1blk, _ = _geom(K)
    sl = np.asarray(w_e)[:, q * hsh:(q + 1) * hsh]         # [C, hsh]
    a = sl.reshape(CC, 128, hshh, 128)                     # [cc, p, hh, f]
    a = a.transpose(1, 2, 0, 3)                            # [p, hh, cc, f]
    return np.ascontiguousarray(a.reshape(128, w1blk)
                                ).astype(ml_dtypes.bfloat16)


def pack_wo(w_e, q, K=KSH):
    """wo [H, C] f32, H-chunk q of K -> [128, w2blk] bf16, co-major."""
    hsh, hshh, _, _, w2blk = _geom(K)
    sl = np.asarray(w_e)[q * hsh:(q + 1) * hsh, :]         # [hsh, C]
    a = sl.reshape(hshh, 128, CO, 128)                     # [hh, p, co, f]
    a = a.transpose(1, 2, 0, 3)                            # [p, co, hh, f]
    return np.ascontiguousarray(a.reshape(128, w2blk)
                                ).astype(ml_dtypes.bfloat16)


def pack_x(x_disp_T, j, cap):
    """x^T slab [C, cap] f32 -> [128, CC*cap] bf16 per slot_tiling(j, cap)."""
    a = x_disp_T.reshape(CC, 128, cap)
    parts = []
    for t0, tw in slot_tiling(j, cap):
        blk = a[:, :, t0:t0 + tw].transpose(1, 0, 2)
        parts.append(blk.reshape(128, CC * tw))
    return np.ascontiguousarray(np.concatenate(parts, axis=1)
                                ).astype(ml_dtypes.bfloat16)


def _route(x, w_gate):
    """Host-side router. Runs the exact reference ops on jax-CPU so the
    top-2 selection and gate values match the reference bit-for-bit."""
    import jax
    import jax.numpy as jnp
    cpu = jax.devices("cpu")[0]
    with jax.default_device(cpu):
        xj = jnp.asarray(np.asarray(x))
        wj = jnp.asarray(np.asarray(w_gate))
        logits = jnp.einsum("btc,ce->bte", xj, wj)
        gates = jax.nn.softmax(logits, axis=-1)
        topk_vals, topk_idx = jax.lax.top_k(gates, TOP_K)
    return (np.asarray(topk_vals).reshape(-1, TOP_K),
            np.asarray(topk_idx).reshape(-1, TOP_K))


def make_in_maps(x, wi, wg, wo, topk_idx, K=KSH):
    """Dispatch + pack per-core inputs.

    Returns (idx_lists, pos, groups, caps, in_maps) where groups[k] lists
    the 8/K experts of slot k (core c runs groups[k][c // K]).
    """
    hsh, hshh, nslot, w1blk, w2blk = _geom(K)
    N = x.shape[0] * x.shape[1] if np.asarray(x).ndim == 3 else x.shape[0]
    x_flat = np.ascontiguousarray(np.asarray(x).reshape(N, C))
    idx_lists = []
    pos = np.empty((N, TOP_K), dtype=np.int64)
    for e in range(N_EXPERTS):
        sel = (topk_idx == e)
        toks = np.flatnonzero(sel.any(axis=1))
        idx_lists.append(toks)
        pos_of = np.full(N, -1, dtype=np.int64)
        pos_of[toks] = np.arange(len(toks))
        for k in range(TOP_K):
            m = sel[:, k]
            pos[m, k] = pos_of[m]

    # group experts by sorted load so each slot's cap (group max) is tight
    gsz = N_EXPERTS // nslot
    order = sorted(range(N_EXPERTS), key=lambda e: -len(idx_lists[e]))
    groups = [order[k * gsz:(k + 1) * gsz] for k in range(nslot)]
    caps = [max(4, -(-max(len(idx_lists[e]) for e in g) // 4) * 4)
            for g in groups]

    xT = np.ascontiguousarray(x_flat.T)

    def xtb_for(experts):
        slabs = []
        for k, e in enumerate(experts):
            toks = idx_lists[e]
            slab = np.zeros((C, caps[k]), dtype=np.float32)
            slab[:, :len(toks)] = xT[:, toks]
            slabs.append(pack_x(slab, k, caps[k]))
        return np.ascontiguousarray(np.concatenate(slabs, axis=1))

    xtbs = [xtb_for([g[side] for g in groups]) for side in range(gsz)]

    in_maps = []
    for c in range(N_CORES):
        side = c // K
        q = c % K
        experts = [g[side] for g in groups]
        in_maps.append({
            "xtb": xtbs[side],
            "wib": np.ascontiguousarray(np.concatenate(
                [pack_wi(wi[e], q, K) for e in experts], axis=1)),
            "wgb": np.ascontiguousarray(np.concatenate(
                [pack_wi(wg[e], q, K) for e in experts], axis=1)),
            "wob": np.ascontiguousarray(np.concatenate(
                [pack_wo(wo[e], q, K) for e in experts], axis=1)),
        })
    return idx_lists, pos, groups, caps, in_maps


def kernel(x, w_gate, wi, wg, wo):
    x = np.asarray(x)
    wi, wg, wo = np.asarray(wi), np.asarray(wg), np.asarray(wo)
    K = KSH

    topk_vals, topk_idx = _route(x, w_gate)
    idx_lists, pos, groups, caps, in_maps = make_in_maps(
        x, wi, wg, wo, topk_idx, K)

    nc = build_program(caps, K)
    res = run_bass_kernel_spmd(nc, in_maps, core_ids=list(range(N_CORES)))

    capoff = np.cumsum([0] + caps[:-1])
    mcap = max(caps)
    # Y[e]: sum of the K H-chunk partials for expert e
    Y = np.zeros((N_EXPERTS, mcap, C), dtype=np.float32)
    for k, g in enumerate(groups):
        for side, e in enumerate(g):
            acc = np.zeros((C, caps[k]), dtype=np.float32)
            for c in range(side * K, side * K + K):
                acc += res.results[c]["ytp"][:,
                                             capoff[k]:capoff[k] + caps[k]
                                             ].astype(np.float32)
            Y[e, :caps[k]] = acc.T
    out = (topk_vals[:, 0:1] * Y[topk_idx[:, 0], pos[:, 0], :]
           + topk_vals[:, 1:2] * Y[topk_idx[:, 1], pos[:, 1], :])
    return out.reshape(B, T, C).astype(np.float32)
